# revision 17
# baseline (speedup 1.0000x reference)
"""Adaptive average pool 2D (16, 256, 224, 224) -> (16, 256, 7, 7) on 8 TRN2 NeuronCores.

224 / 7 = 32 exactly, so every adaptive-pool window is a non-overlapping
32x32 block: out[b, c, o, p] = mean(x[b, c, 32o:32o+32, 32p:32p+32]).

Sharding: pure data parallel over batch — 2 batches (512 (b, c) images) per core.

Per-core design (memory-roofline bound: 102.8 MB of input per core):
  - Each of the 512 images is 224*224 contiguous floats in HBM. A load tile
    holds one 32-row h-block for 128 images: [128 partitions, 32*224] =
    28 KiB contiguous per partition, 3.7 MB per dma_start — all 16 SDMA
    engines run at line rate (~428 GB/s solo, ~358 GB/s when the HBM-stack
    neighbour core is also streaming).
  - One VectorE reduce_sum (axis=XY) per tile computes the 7 window sums of
    each image's h-block; a per-group 1/1024 tensor_scalar produces the mean.
  - Results accumulate in a persistent [128, 4*49] SBUF buffer; each
    128-image group is stored to HBM as soon as it completes, overlapped
    with remaining loads.
  - The last 3 tiles are split into half-row pieces so the VectorE backlog
    (7.6 us reduce vs 8.6 us load interval) is drained by the time the final
    byte lands; only ~4 us of reduce remains on the tail. Half results go to
    scratch and are combined with one add.

Raw Bass (no Tile framework):
  - The walrus DMA lowering accepts only ONE sync-wait per DMACopy, so all
    waits are standalone sequencer waits and each DMA carries exactly one
    semaphore update.
  - Every slot use has exactly one DMA incrementing that slot's semaphore;
    two DMAs may never share a semaphore concurrently (per-engine +1s from
    different DMAs interleave, so a shared count can hit the threshold
    before the first DMA fully landed).
  - Raw bass emits no per-op DRAIN: consecutive VectorE ops have a real RAW
    pipeline hazard (op N+1 reads before op N's tail writes land). Every
    same-engine RAW pair is guarded with an explicit vector.drain().
"""

import numpy as np
from contextlib import ExitStack

import concourse.bass as bass
from concourse import mybir
from concourse.bass_utils import run_bass_kernel_spmd

N_CORES = 8
B, C, H, W = 16, 256, 224, 224
HO = WO = 7
BH, BW = H // HO, W // WO            # 32, 32
IMGS = (B // N_CORES) * C            # 512 images per core
PG = IMGS // 128                     # 4 partition groups of 128 images
ROW = BH * W                         # 7168 floats per (image, h-block)
NTILES = PG * HO                     # 28 (group, h-block) tiles per core
NBUF = 4                             # load slots (ROW-sized)
NSPLIT = 3                           # trailing tiles split into half-rows
HALF = ROW // 2
SCALE = 1.0 / (BH * BW)

_CACHE = {}


def build_nc(nbuf=NBUF, nsplit=NSPLIT):
    nc = bass.Bass("TRN2", debug=False, num_devices=N_CORES)
    x = nc.dram_tensor("x", [IMGS, H * W], mybir.dt.float32, kind="ExternalInput")
    out = nc.dram_tensor("out", [IMGS, HO * WO], mybir.dt.float32, kind="ExternalOutput")
    xa, oa = x.ap(), out.ap()
    oav = oa.rearrange("(g p) j -> p g j", g=PG)   # [128, group, 49]

    # pieces: (tile, lo, ln); full tiles then half-row pieces for the tail
    pieces = [(t, 0, ROW) for t in range(NTILES - nsplit)]
    for t in range(NTILES - nsplit, NTILES):
        pieces += [(t, 0, HALF), (t, HALF, HALF)]
    NP = len(pieces)
    # piece index after which group g's store can be emitted: the load gated
    # on the group's last reduce
    last_piece_of_tile = {t: max(k for k, p in enumerate(pieces) if p[0] == t) for t in range(NTILES)}
    store_after = {min(last_piece_of_tile[7 * g + 6] + nbuf, NP - 1): g for g in range(PG - 1)}

    with ExitStack() as ctx:
        tiles = ctx.enter_context(nc.sbuf_tensor([128, nbuf * ROW], mybir.dt.float32))
        ob = ctx.enter_context(nc.sbuf_tensor([128, PG * HO * WO], mybir.dt.float32))
        scratch = ctx.enter_context(nc.sbuf_tensor([128, 2 * WO], mybir.dt.float32))
        slot_sem = [ctx.enter_context(nc.semaphore(f"slot{j}")) for j in range(nbuf)]
        red_done = ctx.enter_context(nc.semaphore("red_done"))
        grp_done = ctx.enter_context(nc.semaphore("grp_done"))
        out_sem = ctx.enter_context(nc.semaphore("out_sem"))
        block = ctx.enter_context(nc.Block())

        def src(k):
            t, lo, ln = pieces[k]
            g, hb = divmod(t, HO)
            return xa[g * 128:(g + 1) * 128, hb * ROW + lo:hb * ROW + lo + ln]

        def slot(k):
            j = k % nbuf
            ln = pieces[k][2]
            return tiles[:, j * ROW:j * ROW + ln]

        def emit_store(sync, g):
            sync.wait_ge(grp_done, g + 1)
            sync.dma_start(
                out=oav[:, g:g + 1, :],
                in_=ob.ap()[:, g * HO * WO:(g + 1) * HO * WO].rearrange("p (o j) -> p o j", o=1),
            ).then_inc(out_sem, 16)

        @block.sync
        def _(sync):
            for k in range(NP):
                if k >= nbuf:
                    # slot reuse: wait until the reduce that read this slot ran
                    sync.wait_ge(red_done, k - nbuf + 1)
                sync.dma_start(out=slot(k), in_=src(k)).then_inc(slot_sem[k % nbuf], 16)
                if k in store_after:
                    emit_store(sync, store_after[k])
            emit_store(sync, PG - 1)
            sync.wait_ge(out_sem, 16 * PG)

        @block.vector
        def _(vector):
            for k in range(NP):
                t, lo, ln = pieces[k]
                g, hb = divmod(t, HO)
                col = g * HO * WO + hb * WO
                vector.wait_ge(slot_sem[k % nbuf], 16 * (k // nbuf + 1))
                tv = slot(k).rearrange(
                    "p (h pw wi) -> p pw h wi", h=ln // W, pw=WO, wi=BW
                )
                if ln == ROW:
                    vector.reduce_sum(
                        out=ob[:, col:col + WO], in_=tv, axis=mybir.AxisListType.XY
                    ).then_inc(red_done, 1)
                else:
                    h = lo // HALF  # 0 or 1
                    ins = vector.reduce_sum(
                        out=scratch[:, h * WO:(h + 1) * WO],
                        in_=tv,
                        axis=mybir.AxisListType.XY,
                    )
                    ins.then_inc(red_done, 1)
                    if h == 1:
                        vector.drain()  # RAW: half B's tail writes to scratch
                        vector.tensor_add(
                            ob[:, col:col + WO], scratch[:, :WO], scratch[:, WO:]
                        )
                if hb == HO - 1 and lo + ln == ROW:
                    vector.drain()  # RAW: the group's last reduce/add tail writes
                    vector.tensor_scalar_mul(
                        ob[:, g * HO * WO:(g + 1) * HO * WO],
                        ob[:, g * HO * WO:(g + 1) * HO * WO],
                        SCALE,
                    ).then_inc(grp_done, 1)

    return nc


def get_nc():
    if "nc" not in _CACHE:
        _CACHE["nc"] = build_nc()
    return _CACHE["nc"]


def shard_inputs(x):
    x = np.asarray(x, dtype=np.float32).reshape(N_CORES, IMGS, H * W)
    return [{"x": np.ascontiguousarray(x[i])} for i in range(N_CORES)]


def kernel(x, H_in=224, W_in=224, **_):
    assert int(H_in) == H and int(W_in) == W
    res = run_bass_kernel_spmd(get_nc(), shard_inputs(x), core_ids=list(range(N_CORES)))
    out = np.stack([np.asarray(res.results[i]["out"]) for i in range(N_CORES)])
    return out.reshape(B, C, HO, WO)
